# revision 29
# baseline (speedup 1.0000x reference)
"""Trainium2 Bass kernel for nn_ActivatedHeteroLinear (moe_routing, 8 cores).

Math: per type t in {user, item}:
    h = (x @ W1 + b1) @ W2 + b2 = x @ Wc + c        (Wc = W1@W2)
    BatchNorm (training mode) is shift-invariant -> the bias c cancels.
    out = LeakyReLU(a * u + b) = LeakyReLU(x @ W' + b),
    W' = Wc * a (columns scaled), a = gamma*rsqrt(var+eps), b = beta - mean*a
    mean = (m @ Wc)/N with m = sum_rows(x); E[u^2] = diag(Wc^T G Wc)/N with
    G = x^T x; var = E[u^2] - mean^2.  Sync-BN = one 66KB AllReduce of
    [G | m] per type.

Schedule:
    setup Wc -> item-p1 (G, m, SBUF-resident xT store bf16)
             -> kick AR_item -> user-pA (G, m only; overlaps AR_item)
             -> kick AR_user -> stats (row layout; fold a into W', b into a
                bias row) -> item-p2 interleaved with user-pB.

p1 per [128 rows, 128 feat] subtile: DMA-cast x f32->bf16 in 1MB chunks
    (packed: partition p holds qc consecutive rows); G += x^T x (PE);
    xT = transpose(x) (PE -> bf16 psum); one ones^T @ slab matmul per
    4-subtile group accumulates m; evict xT -> xT store (DVE).
p2 per 4-subtile group (two-stage chain, natural layout, NO transpose-back):
    psum <- bias row b broadcast (one K=1 matmul, start=True)
    psum += xT_s^T @ W'  per subtile (N=128, accumulate)
    stg  <- LeakyReLU(psum)   (one ScalarE Lrelu = the eviction itself)
    DMA out (contiguous).
user pB re-reads x_user, re-transposes, then the same natural-layout stage;
its even/odd pair halves write interleaved 128-col staging blocks.

x_user [50000,64]/core is viewed host-side as row pairs [25000,128] so both
types share the d=128 path; user W' is stacked [W';W'] and its natural
matmuls use K=64 halves; the pair-G/pair-m fold after the AllReduce.
Rows are zero-padded host-side to a multiple of 128 (zero rows contribute
nothing to G or m).
"""
import sys

for _p in ("/opt/trn_rl_repo",):
    if _p not in sys.path:
        sys.path.insert(0, _p)

import numpy as np

import concourse.mybir as mybir
import concourse.tile as tile
from concourse import bacc
from concourse.masks import make_identity
from concourse.bass_utils import run_bass_kernel_spmd

F32 = mybir.dt.float32
BF16 = mybir.dt.bfloat16
ALU = mybir.AluOpType
AFT = mybir.ActivationFunctionType

NCORES = 8
EPS = 1e-5
NEG_SLOPE = 0.01

N_USER, N_ITEM = 400000, 600000
HID, D_OUT = 256, 128

CHUNK_Q = 16          # packed subtiles per input DMA chunk (1 MB f32)
P2_START_PURE = 6     # item-p2 chunks before user-pB interleave starts


def _ceil_to(x, m):
    return (x + m - 1) // m * m


class TypeCfg:
    def __init__(self, name, n_rows_core, d_in, n_total_rows):
        self.name = name
        self.d_in = d_in                        # true d_in: 64 or 128
        self.paired = d_in == 64
        self.rpp = 2 if self.paired else 1      # real rows per packed row
        assert n_rows_core % self.rpp == 0
        self.Np = _ceil_to(n_rows_core // self.rpp, 128)  # padded packed rows
        self.n_rows_core = n_rows_core
        self.n_out_pad = self.Np * self.rpp
        self.N_total = n_total_rows


def _chunks(Np):
    nsub = Np // 128
    out, s = [], 0
    while s < nsub:
        qc = min(CHUNK_Q, nsub - s)
        out.append((s, qc))
        s += qc
    return out


class _Ctx:
    pass


def build_kernel(user, item, ncores=NCORES):
    nc = bacc.Bacc(None, target_bir_lowering=False, num_devices=ncores)
    cfgs = [user, item]

    ext = {}
    for c in cfgs:
        ext[c.name] = {
            "x": nc.declare_dram_parameter(f"x_{c.name}", [c.Np, 128], F32, isOutput=False),
            "W1": nc.declare_dram_parameter(f"W1_{c.name}", [c.d_in, HID], F32, isOutput=False),
            "W2": nc.declare_dram_parameter(f"W2_{c.name}", [HID, D_OUT], F32, isOutput=False),
            "gamma": nc.declare_dram_parameter(f"gamma_{c.name}", [D_OUT], F32, isOutput=False),
            "beta": nc.declare_dram_parameter(f"beta_{c.name}", [D_OUT], F32, isOutput=False),
            "out": nc.declare_dram_parameter(f"out_{c.name}", [c.n_out_pad, D_OUT], F32, isOutput=True),
        }
    ar_in = {c.name: nc.dram_tensor(f"ar_in_{c.name}", [128, 129], F32) for c in cfgs}
    ar_out = {c.name: nc.dram_tensor(f"ar_out_{c.name}", [128, 129], F32, addr_space="Shared")
              for c in cfgs}
    g = _Ctx()
    g.nc, g.rg = nc, [list(range(ncores))]

    with tile.TileContext(nc) as tc:
        g.tc = tc
        with tc.tile_pool(name="const", bufs=1) as constp:
            g.constp = constp
            ident_bf = constp.tile([128, 128], BF16)
            make_identity(nc, ident_bf[:])
            ident_f32 = constp.tile([128, 128], F32)
            make_identity(nc, ident_f32[:])
            ones_f32 = constp.tile([128, 1], F32)
            nc.gpsimd.memset(ones_f32[:], 1.0)
            ones_bf = constp.tile([128, 1], BF16)
            nc.gpsimd.memset(ones_bf[:], 1.0)
            ones_row_bf = constp.tile([1, 128], BF16)
            nc.gpsimd.memset(ones_row_bf[:], 1.0)
            ones_row_f32 = constp.tile([1, 128], F32)
            nc.gpsimd.memset(ones_row_f32[:], 1.0)
            eps_v = constp.tile([1, 1], F32)
            nc.gpsimd.memset(eps_v[:], EPS)
            zero_v = constp.tile([1, 1], F32)
            nc.gpsimd.memset(zero_v[:], 0.0)
            zero_col = constp.tile([128, 1], F32)
            nc.gpsimd.memset(zero_col[:], 0.0)
            g.ident_bf, g.ident_f32 = ident_bf, ident_f32
            g.ones_f32, g.ones_bf = ones_f32, ones_bf
            g.ones_row_bf, g.ones_row_f32 = ones_row_bf, ones_row_f32
            g.eps_v, g.zero_v, g.zero_col = eps_v, zero_v, zero_col

            # ---- setup: Wc = W1 @ W2 per type ----
            g.wc_f32 = {}
            with (
                tc.tile_pool(name="wsetup", bufs=1) as wsp,
                tc.tile_pool(name="wps", bufs=1, space="PSUM") as wps,
            ):
                for c in cfgs:
                    d = c.d_in
                    w1 = wsp.tile([d, HID], F32, tag=f"w1_{c.name}")
                    nc.sync.dma_start(w1[:], ext[c.name]["W1"][:])
                    w2a = wsp.tile([128, D_OUT], F32, tag=f"w2a_{c.name}")
                    w2b = wsp.tile([128, D_OUT], F32, tag=f"w2b_{c.name}")
                    nc.sync.dma_start(w2a[:], ext[c.name]["W2"][0:128, :])
                    nc.sync.dma_start(w2b[:], ext[c.name]["W2"][128:256, :])
                    wc_ps = wps.tile([d, D_OUT], F32, tag="wc")
                    for h, w2h in enumerate((w2a, w2b)):
                        w1t_ps = wps.tile([128, d], F32, tag="w1t")
                        nc.tensor.transpose(w1t_ps[:], w1[:, h * 128:(h + 1) * 128],
                                            ident_f32[0:d, 0:d])
                        w1t = wsp.tile([128, d], F32, tag=f"w1t_{c.name}_{h}")
                        nc.vector.tensor_copy(w1t[:], w1t_ps[:])
                        nc.tensor.matmul(wc_ps[:], w1t[:], w2h[:],
                                         start=(h == 0), stop=(h == 1))
                    wf = constp.tile([d, D_OUT], F32, tag=f"wcf_{c.name}")
                    nc.vector.tensor_copy(wf[:], wc_ps[:])
                    g.wc_f32[c.name] = wf

            _phases(g, user, item, ext, ar_in, ar_out)

    nc.finalize()
    return nc


def _phases(g, user, item, ext, ar_in, ar_out):
    nc, tc = g.nc, g.tc
    with (
        tc.tile_pool(name="xt", bufs=1) as xtp,
        tc.tile_pool(name="xchp", bufs=3) as xchp,
        tc.tile_pool(name="smp", bufs=1) as smp,
    ):
        g.smp, g.xchp = smp, xchp
        xt_item = xtp.tile([128, item.Np], BF16)

        with (
            tc.tile_pool(name="psA", bufs=4, space="PSUM") as psA,
            tc.tile_pool(name="psG", bufs=1, space="PSUM") as psG,
        ):
            g.psA, g.psG = psA, psG
            _p1(g, item, ext[item.name], xt_item)
            _kick_ar(g, item, ar_in[item.name])
            nc.gpsimd.collective_compute("AllReduce", ALU.add, replica_groups=g.rg,
                                         ins=[ar_in[item.name][:]],
                                         outs=[ar_out[item.name][:]])

            _p1(g, user, ext[user.name], None)
            _kick_ar(g, user, ar_in[user.name])
            nc.gpsimd.collective_compute("AllReduce", ALU.add, replica_groups=g.rg,
                                         ins=[ar_in[user.name][:]],
                                         outs=[ar_out[user.name][:]])

            wp_item = _stats(g, item, ext[item.name], ar_out[item.name])
            wp_user = _stats(g, user, ext[user.name], ar_out[user.name])

        # ---- p2: decoupled pools per stream ----
        with (
            tc.tile_pool(name="p2i", bufs=2) as p2i,
            tc.tile_pool(name="p2u", bufs=2) as p2u,
            tc.tile_pool(name="psI", bufs=3, space="PSUM") as psI,
            tc.tile_pool(name="psUx", bufs=2, space="PSUM") as psUx,
            tc.tile_pool(name="psUn", bufs=3, space="PSUM") as psUn,
        ):
            g.p2i, g.p2u = p2i, p2u
            g.psI, g.psUx, g.psUn = psI, psUx, psUn
            ig = _p2_item_gen(g, item, ext[item.name], xt_item, wp_item)
            ug = _pb_user_gen(g, user, ext[user.name], wp_user)
            for _ in range(4 * P2_START_PURE):
                next(ig, None)
            alive = True
            while alive:
                alive = False
                if next(ig, None) is not None:
                    alive = True
                if next(ug, None) is not None:
                    alive = True


def _p1(g, c, ex, xt_store):
    """G + m accumulation; if xt_store is given, also transpose + store xT.
    Two-group software-pipeline skew keeps the PE stream dense."""
    nc = g.nc
    chunks = _chunks(c.Np)
    nsub = c.Np // 128
    n_groups = sum((qc + 3) // 4 for _, qc in chunks)
    full = xt_store is not None
    gm = g.psG.tile([128, 128], F32, tag="gbank")
    g.g_bank = getattr(g, "g_bank", {})
    g.g_bank[c.name] = gm
    g.m_bank = getattr(g, "m_bank", {})
    m_ps = g.psG.tile([1, 512], F32, tag="mps")
    g.m_bank[c.name] = m_ps

    sub_i = 0
    grp_i = 0
    for s0, qc in chunks:
        xch = g.xchp.tile([128, CHUNK_Q * 128], BF16, tag="xch")
        src = ex["x"][s0 * 128:(s0 + qc) * 128, :].rearrange(
            "(p q) d -> p (q d)", p=128)
        nc.gpsimd.dma_start(xch[:, 0:qc * 128], src)   # f32 -> bf16 cast
        for g0 in range(0, qc, 4):
            ns = min(4, qc - g0)
            slab = xch[:, g0 * 128:(g0 + ns) * 128]
            for i in range(ns):
                sl = xch[:, (g0 + i) * 128:(g0 + i + 1) * 128]
                nc.tensor.matmul(gm[:], sl, sl,
                                 start=(sub_i == 0), stop=(sub_i == nsub - 1),
                                 skip_group_check=True)
                sub_i += 1
            nc.tensor.matmul(m_ps[0:1, 0:ns * 128], g.ones_bf[:], slab,
                             start=(grp_i == 0), stop=(grp_i == n_groups - 1),
                             skip_group_check=True)
            grp_i += 1
            if full:
                blk0 = s0 + g0
                dst = xt_store[:, blk0 * 128:(blk0 + ns) * 128].rearrange(
                    "p (n f) -> p n f", f=128)
                nc.sync.dma_start_transpose(dst, slab)
    assert sub_i == nsub and grp_i == n_groups


def _kick_ar(g, c, ar_in):
    nc = g.nc
    gm = g.g_bank[c.name]
    g_sb = g.smp.tile([128, 128], F32, tag=f"gsb_{c.name}")
    nc.vector.tensor_copy(g_sb[:], gm[:])
    nc.sync.dma_start(ar_in[:, 0:128], g_sb[:])
    # fold the [1,512] m blocks, transpose to a column
    m_row4 = g.smp.tile([1, 512], F32, tag=f"mrow4_{c.name}")
    nc.vector.tensor_copy(m_row4[:], g.m_bank[c.name][:])
    m_row = g.smp.tile([1, 128], F32, tag=f"mrow_{c.name}")
    nc.vector.tensor_tensor(m_row[:], m_row4[:, 0:128], m_row4[:, 128:256], ALU.add)
    nc.vector.tensor_tensor(m_row[:], m_row[:], m_row4[:, 256:384], ALU.add)
    nc.vector.tensor_tensor(m_row[:], m_row[:], m_row4[:, 384:512], ALU.add)
    mt_ps = g.psG.tile([128, 4], F32, tag="mps")
    nc.tensor.transpose(mt_ps[:, 0:1], m_row[:], g.ident_f32[0:1, 0:1])
    m_col = g.smp.tile([128, 1], F32, tag=f"mcol_{c.name}")
    nc.vector.tensor_copy(m_col[:], mt_ps[:, 0:1])
    nc.sync.dma_start(ar_in[:, 128:129], m_col[:])


def _stats(g, c, ex, ar_out):
    """AllReduced [G|m] -> (W'_stack bf16 [128,128], b_row4 bf16 [1,512]).
    All per-column stats math in row layout on partition 0."""
    nc, d = g.nc, c.d_in
    smp, psG = g.smp, g.psG
    name = c.name
    ar_sb = smp.tile([128, 129], F32, tag=f"arsb_{name}")
    nc.sync.dma_start(ar_sb[:], ar_out[:])

    if c.paired:
        tmp = smp.tile([64, 65], F32, tag=f"fold_{name}")
        nc.sync.dma_start(tmp[:], ar_sb[64:128, 64:129])   # cross-partition
        g_eff = smp.tile([64, 64], F32, tag=f"geff_{name}")
        nc.vector.tensor_tensor(g_eff[:], ar_sb[0:64, 0:64], tmp[:, 0:64], ALU.add)
        m_eff = smp.tile([64, 1], F32, tag=f"meff_{name}")
        nc.vector.tensor_tensor(m_eff[:], ar_sb[0:64, 128:129], tmp[:, 64:65],
                                ALU.add)
        g_ap, m_ap = g_eff[:], m_eff[:]
    else:
        g_ap, m_ap = ar_sb[:, 0:128], ar_sb[:, 128:129]

    wc = g.wc_f32[name]
    t1_ps = psG.tile([d, D_OUT], F32, tag="mps")
    nc.tensor.matmul(t1_ps[:], g_ap, wc[:], start=True, stop=True)
    t1 = smp.tile([d, D_OUT], F32, tag=f"t1_{name}")
    nc.vector.tensor_copy(t1[:], t1_ps[:])
    t2 = smp.tile([d, D_OUT], F32, tag=f"t2_{name}")
    nc.vector.tensor_tensor(t2[:], t1[:], wc[:], ALU.mult)
    e2_ps = psG.tile([1, D_OUT], F32, tag="mps")
    nc.tensor.matmul(e2_ps[:], g.ones_f32[0:d, :], t2[:], start=True, stop=True)
    e2_row = smp.tile([1, D_OUT], F32, tag=f"e2_{name}")
    nc.vector.tensor_copy(e2_row[:], e2_ps[:])
    s_ps = psG.tile([1, D_OUT], F32, tag="mps")
    nc.tensor.matmul(s_ps[:], m_ap, wc[:], start=True, stop=True)
    s_row = smp.tile([1, D_OUT], F32, tag=f"s_{name}")
    nc.vector.tensor_copy(s_row[:], s_ps[:])
    gamma_row = smp.tile([1, D_OUT], F32, tag=f"gam_{name}")
    nc.sync.dma_start(gamma_row[:], ex["gamma"][:].rearrange("(o f) -> o f", o=1))
    beta_row = smp.tile([1, D_OUT], F32, tag=f"bet_{name}")
    nc.sync.dma_start(beta_row[:], ex["beta"][:].rearrange("(o f) -> o f", o=1))

    inv_n = 1.0 / float(c.N_total)
    mean = smp.tile([1, D_OUT], F32, tag=f"mean_{name}")
    nc.vector.tensor_scalar(mean[:], s_row[:], inv_n, None, ALU.mult)
    msq = smp.tile([1, D_OUT], F32, tag=f"msq_{name}")
    nc.vector.tensor_tensor(msq[:], mean[:], mean[:], ALU.mult)
    var = smp.tile([1, D_OUT], F32, tag=f"var_{name}")
    nc.vector.tensor_scalar(var[:], e2_row[:], inv_n, None, ALU.mult)
    nc.vector.tensor_tensor(var[:], var[:], msq[:], ALU.subtract)
    lnv = smp.tile([1, D_OUT], F32, tag=f"lnv_{name}")
    nc.scalar.activation(lnv[:], var[:], AFT.Ln, bias=g.eps_v[:], scale=1.0)
    rstd = smp.tile([1, D_OUT], F32, tag=f"rstd_{name}")
    nc.scalar.activation(rstd[:], lnv[:], AFT.Exp, bias=g.zero_v[:], scale=-0.5)
    a_row = smp.tile([1, D_OUT], F32, tag=f"arow_{name}")
    nc.vector.tensor_tensor(a_row[:], gamma_row[:], rstd[:], ALU.mult)
    ma = smp.tile([1, D_OUT], F32, tag=f"ma_{name}")
    nc.vector.tensor_tensor(ma[:], mean[:], a_row[:], ALU.mult)
    b_row = smp.tile([1, D_OUT], F32, tag=f"brow_{name}")
    nc.vector.tensor_tensor(b_row[:], beta_row[:], ma[:], ALU.subtract)

    # a replicated down d partitions via a K=1 outer product; W' = Wc * a
    arep_ps = psG.tile([d, D_OUT], F32, tag="mps")
    nc.tensor.matmul(arep_ps[:], g.ones_row_f32[0:1, 0:d], a_row[:],
                     start=True, stop=True)
    arep = smp.tile([d, D_OUT], F32, tag=f"arep_{name}")
    nc.vector.tensor_copy(arep[:], arep_ps[:])
    wprime = smp.tile([d, D_OUT], F32, tag=f"wp_{name}")
    nc.vector.tensor_tensor(wprime[:], wc[:], arep[:], ALU.mult)
    wp_stack = g.constp.tile([128, D_OUT], BF16, tag=f"wps_{name}")
    nc.vector.tensor_copy(wp_stack[0:d, :], wprime[:])
    if c.paired:
        nc.sync.dma_start(wp_stack[64:128, :], wp_stack[0:64, :])
    # bias row replicated x4 (bf16) for the K=1 bias-preload matmul
    b_row4 = g.constp.tile([1, 512], BF16, tag=f"br4_{name}")
    for k in range(4):
        nc.vector.tensor_copy(b_row4[:, k * 128:(k + 1) * 128], b_row[:])
    return wp_stack, b_row4


def _p2_item_gen(g, c, ex, xt_store, wp):
    """natural-layout output: psum <- b (K=1 matmul); += x @ W'; Lrelu-evict."""
    nc = g.nc
    wp_stack, b_row4 = wp
    for s0, qc in _chunks(c.Np):
        for z0 in range(0, qc, 8):
            zn = min(8, qc - z0)
            stg = g.p2i.tile([128, 1024], F32, tag="stg_i")
            for t0 in range(0, zn, 4):
                tn = min(4, zn - t0)
                blk0 = s0 + z0 + t0
                nat_ps = g.psI.tile([128, 512], F32, tag="nat_i")
                nc.tensor.matmul(nat_ps[:, 0:tn * 128], g.ones_row_bf[:],
                                 b_row4[:, 0:tn * 128], start=True, stop=False,
                                 skip_group_check=True)
                for i in range(tn):
                    nc.tensor.matmul(
                        nat_ps[:, i * 128:(i + 1) * 128],
                        xt_store[:, (blk0 + i) * 128:(blk0 + i + 1) * 128],
                        wp_stack[:], start=False, stop=True,
                        skip_group_check=True)
                nc.scalar.activation(stg[:, t0 * 128:(t0 + tn) * 128],
                                     nat_ps[:, 0:tn * 128], AFT.Lrelu,
                                     bias=g.zero_col[:], scale=1.0,
                                     alpha=NEG_SLOPE)
                yield True
            dst = ex["out"][s0 * 128:(s0 + qc) * 128, :].rearrange(
                "(p q) e -> p (q e)", p=128)[:, z0 * 128:z0 * 128 + zn * 128]
            nc.sync.dma_start(dst, stg[:, 0:zn * 128])


def _pb_user_gen(g, c, ex, wp):
    """user pass B: re-read x, transpose, natural-layout stage per pair half."""
    nc = g.nc
    wp_stack, b_row4 = wp
    for s0, qc in _chunks(c.Np):
        xch = g.xchp.tile([128, CHUNK_Q * 128], BF16, tag="xch")
        src = ex["x"][s0 * 128:(s0 + qc) * 128, :].rearrange(
            "(p q) d -> p (q d)", p=128)
        nc.gpsimd.dma_start(xch[:, 0:qc * 128], src)
        out_rr = ex["out"][s0 * 256:(s0 + qc) * 256, :].rearrange(
            "(p q) e -> p (q e)", p=128)
        for g0 in range(0, qc, 4):
            ns = min(4, qc - g0)
            xt_sb = g.p2u.tile([128, 512], BF16, tag="xtsb_u")
            nc.sync.dma_start_transpose(
                xt_sb[:, 0:ns * 128].rearrange("p (n f) -> p n f", f=128),
                xch[:, g0 * 128:(g0 + ns) * 128])
            stg = g.p2u.tile([128, 1024], F32, tag="stg_u")
            for half in range(2):
                nat_ps = g.psUn.tile([128, 512], F32, tag="nat_u")
                nc.tensor.matmul(nat_ps[:, 0:ns * 128], g.ones_row_bf[:],
                                 b_row4[:, 0:ns * 128], start=True, stop=False,
                                 skip_group_check=True)
                for i in range(ns):
                    nc.tensor.matmul(
                        nat_ps[:, i * 128:(i + 1) * 128],
                        xt_sb[half * 64:(half + 1) * 64, i * 128:(i + 1) * 128],
                        wp_stack[half * 64:(half + 1) * 64, :],
                        start=False, stop=True, skip_group_check=True)
                # this half's real 128-col blocks interleave into staging
                dst = stg[:, 0:2 * ns * 128].rearrange(
                    "p (n two f) -> p n two f", two=2, f=128)[:, :, half, :]
                nc.scalar.activation(
                    dst, nat_ps[:, 0:ns * 128].rearrange("p (n f) -> p n f", f=128),
                    AFT.Lrelu, bias=g.zero_col[:], scale=1.0, alpha=NEG_SLOPE)
            dcols = 2 * ns * 128
            nc.sync.dma_start(out_rr[:, 2 * g0 * 128:2 * g0 * 128 + dcols],
                              stg[:, 0:dcols])
            yield True


# ---------------------------------------------------------------------------
_BUILT = {}


def _get_built():
    if "full" not in _BUILT:
        user = TypeCfg("user", N_USER // NCORES, 64, N_USER)
        item = TypeCfg("item", N_ITEM // NCORES, 128, N_ITEM)
        _BUILT["full"] = (build_kernel(user, item), user, item)
    return _BUILT["full"]


def kernel(x_user, x_item,
           W1_user=None, b1_user=None, W1_item=None, b1_item=None,
           W2_user=None, b2_user=None, W2_item=None, b2_item=None,
           gamma_user=None, beta_user=None, gamma_item=None, beta_item=None,
           _trace=False):
    nc, ucfg, icfg = _get_built()

    def prep(x, cfg):
        x = np.ascontiguousarray(np.asarray(x, np.float32))
        n = x.shape[0] // NCORES
        shards = []
        for i in range(NCORES):
            s = x[i * n:(i + 1) * n].reshape(-1, 128)
            pad = cfg.Np - s.shape[0]
            if pad:
                s = np.concatenate([s, np.zeros((pad, 128), np.float32)], 0)
            shards.append(s)
        return shards

    xu = prep(x_user, ucfg)
    xi = prep(x_item, icfg)
    common = {
        "W1_user": np.asarray(W1_user, np.float32),
        "W2_user": np.asarray(W2_user, np.float32),
        "gamma_user": np.asarray(gamma_user, np.float32),
        "beta_user": np.asarray(beta_user, np.float32),
        "W1_item": np.asarray(W1_item, np.float32),
        "W2_item": np.asarray(W2_item, np.float32),
        "gamma_item": np.asarray(gamma_item, np.float32),
        "beta_item": np.asarray(beta_item, np.float32),
    }
    in_maps = [dict(common, x_user=xu[i], x_item=xi[i]) for i in range(NCORES)]
    res = run_bass_kernel_spmd(nc, in_maps, list(range(NCORES)), trace=_trace)
    nu, ni = N_USER // NCORES, N_ITEM // NCORES
    out_user = np.concatenate(
        [res.results[i]["out_user"][:nu] for i in range(NCORES)], 0)
    out_item = np.concatenate(
        [res.results[i]["out_item"][:ni] for i in range(NCORES)], 0)
    if _trace:
        kernel.last_exec_time_ns = res.exec_time_ns
    return (out_user, out_item)


# revision 31
# speedup vs baseline: 1.5834x; 1.5834x over previous
"""Trainium2 Bass kernel for nn_ActivatedHeteroLinear (moe_routing, 8 cores).

Math: per type t in {user, item}:
    h = (x @ W1 + b1) @ W2 + b2 = x @ Wc + c        (Wc = W1@W2)
    BatchNorm (training mode) is shift-invariant -> the bias c cancels.
    out = LeakyReLU(a * u + b) = LeakyReLU(x @ W' + b),
    W' = Wc * a (columns scaled), a = gamma*rsqrt(var+eps), b = beta - mean*a
    mean = (m @ Wc)/N with m = sum_rows(x); E[u^2] = diag(Wc^T G Wc)/N with
    G = x^T x; var = E[u^2] - mean^2.  Sync-BN = one 66KB AllReduce of
    [G | m] per type.

Schedule:
    setup Wc -> item-p1 (G, m, SBUF-resident xT store bf16)
             -> kick AR_item -> user-pA (G, m only; overlaps AR_item)
             -> kick AR_user -> stats (row layout; fold a into W', b into a
                bias row) -> item-p2 interleaved with user-pB.

p1 per [128 rows, 128 feat] subtile: DMA-cast x f32->bf16 in 1MB chunks
    (packed: partition p holds qc consecutive rows); G += x^T x (PE);
    xT = transpose(x) (PE -> bf16 psum); one ones^T @ slab matmul per
    4-subtile group accumulates m; evict xT -> xT store (DVE).
p2 per 4-subtile group (two-stage chain, natural layout, NO transpose-back):
    psum <- bias row b broadcast (one K=1 matmul, start=True)
    psum += xT_s^T @ W'  per subtile (N=128, accumulate)
    stg  <- LeakyReLU(psum)   (one ScalarE Lrelu = the eviction itself)
    DMA out (contiguous).
user pB re-reads x_user, re-transposes, then the same natural-layout stage;
its even/odd pair halves write interleaved 128-col staging blocks.

x_user [50000,64]/core is viewed host-side as row pairs [25000,128] so both
types share the d=128 path; user W' is stacked [W';W'] and its natural
matmuls use K=64 halves; the pair-G/pair-m fold after the AllReduce.
Rows are zero-padded host-side to a multiple of 128 (zero rows contribute
nothing to G or m).
"""
import sys

for _p in ("/opt/trn_rl_repo",):
    if _p not in sys.path:
        sys.path.insert(0, _p)

import numpy as np

import concourse.mybir as mybir
import concourse.tile as tile
from concourse import bacc
from concourse.masks import make_identity
from concourse.bass_utils import run_bass_kernel_spmd

F32 = mybir.dt.float32
BF16 = mybir.dt.bfloat16
ALU = mybir.AluOpType
AFT = mybir.ActivationFunctionType

NCORES = 8
EPS = 1e-5
NEG_SLOPE = 0.01

N_USER, N_ITEM = 400000, 600000
HID, D_OUT = 256, 128

CHUNK_Q = 16          # packed subtiles per input DMA chunk (1 MB f32)
P2_START_PURE = 6     # item-p2 chunks before user-pB interleave starts


def _ceil_to(x, m):
    return (x + m - 1) // m * m


class TypeCfg:
    def __init__(self, name, n_rows_core, d_in, n_total_rows):
        self.name = name
        self.d_in = d_in                        # true d_in: 64 or 128
        self.paired = d_in == 64
        self.rpp = 2 if self.paired else 1      # real rows per packed row
        assert n_rows_core % self.rpp == 0
        self.Np = _ceil_to(n_rows_core // self.rpp, 128)  # padded packed rows
        self.n_rows_core = n_rows_core
        self.n_out_pad = self.Np * self.rpp
        self.N_total = n_total_rows


def _chunks(Np):
    nsub = Np // 128
    out, s = [], 0
    while s < nsub:
        qc = min(CHUNK_Q, nsub - s)
        out.append((s, qc))
        s += qc
    return out


class _Ctx:
    pass


def build_kernel(user, item, ncores=NCORES):
    nc = bacc.Bacc(None, target_bir_lowering=False, num_devices=ncores)
    cfgs = [user, item]

    ext = {}
    for c in cfgs:
        ext[c.name] = {
            "x": nc.declare_dram_parameter(f"x_{c.name}", [c.Np, 128], F32, isOutput=False),
            "W1": nc.declare_dram_parameter(f"W1_{c.name}", [c.d_in, HID], F32, isOutput=False),
            "W2": nc.declare_dram_parameter(f"W2_{c.name}", [HID, D_OUT], F32, isOutput=False),
            "gamma": nc.declare_dram_parameter(f"gamma_{c.name}", [D_OUT], F32, isOutput=False),
            "beta": nc.declare_dram_parameter(f"beta_{c.name}", [D_OUT], F32, isOutput=False),
            "out": nc.declare_dram_parameter(f"out_{c.name}", [c.n_out_pad, D_OUT], F32, isOutput=True),
        }
    ar_in = {c.name: nc.dram_tensor(f"ar_in_{c.name}", [128, 129], F32) for c in cfgs}
    ar_out = {c.name: nc.dram_tensor(f"ar_out_{c.name}", [128, 129], F32, addr_space="Shared")
              for c in cfgs}
    g = _Ctx()
    g.nc, g.rg = nc, [list(range(ncores))]

    with tile.TileContext(nc) as tc:
        g.tc = tc
        with tc.tile_pool(name="const", bufs=1) as constp:
            g.constp = constp
            ident_bf = constp.tile([128, 128], BF16)
            make_identity(nc, ident_bf[:])
            ident_f32 = constp.tile([128, 128], F32)
            make_identity(nc, ident_f32[:])
            ones_f32 = constp.tile([128, 1], F32)
            nc.gpsimd.memset(ones_f32[:], 1.0)
            ones_bf = constp.tile([128, 1], BF16)
            nc.gpsimd.memset(ones_bf[:], 1.0)
            ones_row_bf = constp.tile([1, 128], BF16)
            nc.gpsimd.memset(ones_row_bf[:], 1.0)
            ones_row_f32 = constp.tile([1, 128], F32)
            nc.gpsimd.memset(ones_row_f32[:], 1.0)
            eps_v = constp.tile([1, 1], F32)
            nc.gpsimd.memset(eps_v[:], EPS)
            zero_v = constp.tile([1, 1], F32)
            nc.gpsimd.memset(zero_v[:], 0.0)
            zero_col = constp.tile([128, 1], F32)
            nc.gpsimd.memset(zero_col[:], 0.0)
            g.ident_bf, g.ident_f32 = ident_bf, ident_f32
            g.ones_f32, g.ones_bf = ones_f32, ones_bf
            g.ones_row_bf, g.ones_row_f32 = ones_row_bf, ones_row_f32
            g.eps_v, g.zero_v, g.zero_col = eps_v, zero_v, zero_col

            # ---- setup: Wc = W1 @ W2 per type ----
            g.wc_f32 = {}
            with (
                tc.tile_pool(name="wsetup", bufs=1) as wsp,
                tc.tile_pool(name="wps", bufs=1, space="PSUM") as wps,
            ):
                for c in cfgs:
                    d = c.d_in
                    w1 = wsp.tile([d, HID], F32, tag=f"w1_{c.name}")
                    nc.sync.dma_start(w1[:], ext[c.name]["W1"][:])
                    w2a = wsp.tile([128, D_OUT], F32, tag=f"w2a_{c.name}")
                    w2b = wsp.tile([128, D_OUT], F32, tag=f"w2b_{c.name}")
                    nc.sync.dma_start(w2a[:], ext[c.name]["W2"][0:128, :])
                    nc.sync.dma_start(w2b[:], ext[c.name]["W2"][128:256, :])
                    wc_ps = wps.tile([d, D_OUT], F32, tag="wc")
                    for h, w2h in enumerate((w2a, w2b)):
                        w1t_ps = wps.tile([128, d], F32, tag="w1t")
                        nc.tensor.transpose(w1t_ps[:], w1[:, h * 128:(h + 1) * 128],
                                            ident_f32[0:d, 0:d])
                        w1t = wsp.tile([128, d], F32, tag=f"w1t_{c.name}_{h}")
                        nc.vector.tensor_copy(w1t[:], w1t_ps[:])
                        nc.tensor.matmul(wc_ps[:], w1t[:], w2h[:],
                                         start=(h == 0), stop=(h == 1))
                    wf = constp.tile([d, D_OUT], F32, tag=f"wcf_{c.name}")
                    nc.vector.tensor_copy(wf[:], wc_ps[:])
                    g.wc_f32[c.name] = wf

            _phases(g, user, item, ext, ar_in, ar_out)

    nc.finalize()
    return nc


def _phases(g, user, item, ext, ar_in, ar_out):
    nc, tc = g.nc, g.tc
    with (
        tc.tile_pool(name="xt", bufs=1) as xtp,
        tc.tile_pool(name="xchp", bufs=3) as xchp,
        tc.tile_pool(name="smp", bufs=1) as smp,
    ):
        g.smp, g.xchp = smp, xchp
        xt_item = xtp.tile([128, item.Np], BF16)

        with (
            tc.tile_pool(name="psA", bufs=4, space="PSUM") as psA,
            tc.tile_pool(name="psG", bufs=1, space="PSUM") as psG,
        ):
            g.psA, g.psG = psA, psG
            _p1(g, item, ext[item.name], xt_item)
            _kick_ar(g, item, ar_in[item.name])
            nc.gpsimd.collective_compute("AllReduce", ALU.add, replica_groups=g.rg,
                                         ins=[ar_in[item.name][:]],
                                         outs=[ar_out[item.name][:]])

            _p1(g, user, ext[user.name], None)
            _kick_ar(g, user, ar_in[user.name])
            nc.gpsimd.collective_compute("AllReduce", ALU.add, replica_groups=g.rg,
                                         ins=[ar_in[user.name][:]],
                                         outs=[ar_out[user.name][:]])

            wp_item = _stats(g, item, ext[item.name], ar_out[item.name])
            wp_user = _stats(g, user, ext[user.name], ar_out[user.name])

        # ---- p2: decoupled pools per stream ----
        with (
            tc.tile_pool(name="p2i", bufs=2) as p2i,
            tc.tile_pool(name="p2u", bufs=2) as p2u,
            tc.tile_pool(name="psI", bufs=2, space="PSUM") as psI,
            tc.tile_pool(name="psUx", bufs=2, space="PSUM") as psUx,
            tc.tile_pool(name="psUn", bufs=4, space="PSUM") as psUn,
        ):
            g.p2i, g.p2u = p2i, p2u
            g.psI, g.psUx, g.psUn = psI, psUx, psUn
            ig = _p2_item_gen(g, item, ext[item.name], xt_item, wp_item)
            ug = _pb_user_gen(g, user, ext[user.name], wp_user)
            for _ in range(4 * P2_START_PURE):
                next(ig, None)
            alive = True
            while alive:
                alive = False
                for _ in range(2):
                    if next(ig, None) is not None:
                        alive = True
                if next(ug, None) is not None:
                    alive = True


def _p1(g, c, ex, xt_store):
    """G + m accumulation; if xt_store is given, also transpose + store xT.
    Two-group software-pipeline skew keeps the PE stream dense."""
    nc = g.nc
    chunks = _chunks(c.Np)
    nsub = c.Np // 128
    n_groups = sum((qc + 3) // 4 for _, qc in chunks)
    full = xt_store is not None
    gm = g.psG.tile([128, 128], F32, tag="gbank")
    g.g_bank = getattr(g, "g_bank", {})
    g.g_bank[c.name] = gm
    g.m_bank = getattr(g, "m_bank", {})
    m_ps = g.psG.tile([1, 512], F32, tag="mps")
    g.m_bank[c.name] = m_ps

    def flush(pend):
        xt_ps, ns, blk0 = pend
        nc.vector.tensor_copy(xt_store[:, blk0 * 128:(blk0 + ns) * 128],
                              xt_ps[:, 0:ns * 128])

    sub_i = 0
    grp_i = 0
    pend = []
    for s0, qc in chunks:
        xch = g.xchp.tile([128, CHUNK_Q * 128], BF16, tag="xch")
        src = ex["x"][s0 * 128:(s0 + qc) * 128, :].rearrange(
            "(p q) d -> p (q d)", p=128)
        nc.gpsimd.dma_start(xch[:, 0:qc * 128], src)   # f32 -> bf16 cast
        for g0 in range(0, qc, 4):
            ns = min(4, qc - g0)
            slab = xch[:, g0 * 128:(g0 + ns) * 128]
            sls = [xch[:, (g0 + i) * 128:(g0 + i + 1) * 128] for i in range(ns)]
            if full:
                xt_ps = g.psA.tile([128, 512], BF16, tag="xtnat")
            else:
                xt_ps = None
            for i in range(ns):
                nc.tensor.matmul(gm[:], sls[i], sls[i],
                                 start=(sub_i == 0), stop=(sub_i == nsub - 1),
                                 skip_group_check=True)
                if full:
                    nc.tensor.transpose(xt_ps[:, i * 128:(i + 1) * 128],
                                        sls[i], g.ident_bf[:])
                sub_i += 1
            nc.tensor.matmul(m_ps[0:1, 0:ns * 128], g.ones_bf[:], slab,
                             start=(grp_i == 0), stop=(grp_i == n_groups - 1),
                             skip_group_check=True)
            grp_i += 1
            if not full:
                continue
            if len(pend) == 2:
                flush(pend.pop(0))
            pend.append((xt_ps, ns, s0 + g0))
    for p in pend:
        flush(p)
    assert sub_i == nsub and grp_i == n_groups


def _kick_ar(g, c, ar_in):
    nc = g.nc
    gm = g.g_bank[c.name]
    g_sb = g.smp.tile([128, 128], F32, tag=f"gsb_{c.name}")
    nc.vector.tensor_copy(g_sb[:], gm[:])
    nc.sync.dma_start(ar_in[:, 0:128], g_sb[:])
    # fold the [1,512] m blocks, transpose to a column
    m_row4 = g.smp.tile([1, 512], F32, tag=f"mrow4_{c.name}")
    nc.vector.tensor_copy(m_row4[:], g.m_bank[c.name][:])
    m_row = g.smp.tile([1, 128], F32, tag=f"mrow_{c.name}")
    nc.vector.tensor_tensor(m_row[:], m_row4[:, 0:128], m_row4[:, 128:256], ALU.add)
    nc.vector.tensor_tensor(m_row[:], m_row[:], m_row4[:, 256:384], ALU.add)
    nc.vector.tensor_tensor(m_row[:], m_row[:], m_row4[:, 384:512], ALU.add)
    mt_ps = g.psG.tile([128, 4], F32, tag="mps")
    nc.tensor.transpose(mt_ps[:, 0:1], m_row[:], g.ident_f32[0:1, 0:1])
    m_col = g.smp.tile([128, 1], F32, tag=f"mcol_{c.name}")
    nc.vector.tensor_copy(m_col[:], mt_ps[:, 0:1])
    nc.sync.dma_start(ar_in[:, 128:129], m_col[:])


def _stats(g, c, ex, ar_out):
    """AllReduced [G|m] -> (W'_stack bf16 [128,128], b_row4 bf16 [1,512]).
    All per-column stats math in row layout on partition 0."""
    nc, d = g.nc, c.d_in
    smp, psG = g.smp, g.psG
    name = c.name
    ar_sb = smp.tile([128, 129], F32, tag=f"arsb_{name}")
    nc.sync.dma_start(ar_sb[:], ar_out[:])

    if c.paired:
        tmp = smp.tile([64, 65], F32, tag=f"fold_{name}")
        nc.sync.dma_start(tmp[:], ar_sb[64:128, 64:129])   # cross-partition
        g_eff = smp.tile([64, 64], F32, tag=f"geff_{name}")
        nc.vector.tensor_tensor(g_eff[:], ar_sb[0:64, 0:64], tmp[:, 0:64], ALU.add)
        m_eff = smp.tile([64, 1], F32, tag=f"meff_{name}")
        nc.vector.tensor_tensor(m_eff[:], ar_sb[0:64, 128:129], tmp[:, 64:65],
                                ALU.add)
        g_ap, m_ap = g_eff[:], m_eff[:]
    else:
        g_ap, m_ap = ar_sb[:, 0:128], ar_sb[:, 128:129]

    wc = g.wc_f32[name]
    t1_ps = psG.tile([d, D_OUT], F32, tag="mps")
    nc.tensor.matmul(t1_ps[:], g_ap, wc[:], start=True, stop=True)
    t1 = smp.tile([d, D_OUT], F32, tag=f"t1_{name}")
    nc.vector.tensor_copy(t1[:], t1_ps[:])
    t2 = smp.tile([d, D_OUT], F32, tag=f"t2_{name}")
    nc.vector.tensor_tensor(t2[:], t1[:], wc[:], ALU.mult)
    e2_ps = psG.tile([1, D_OUT], F32, tag="mps")
    nc.tensor.matmul(e2_ps[:], g.ones_f32[0:d, :], t2[:], start=True, stop=True)
    e2_row = smp.tile([1, D_OUT], F32, tag=f"e2_{name}")
    nc.vector.tensor_copy(e2_row[:], e2_ps[:])
    s_ps = psG.tile([1, D_OUT], F32, tag="mps")
    nc.tensor.matmul(s_ps[:], m_ap, wc[:], start=True, stop=True)
    s_row = smp.tile([1, D_OUT], F32, tag=f"s_{name}")
    nc.vector.tensor_copy(s_row[:], s_ps[:])
    gamma_row = smp.tile([1, D_OUT], F32, tag=f"gam_{name}")
    nc.sync.dma_start(gamma_row[:], ex["gamma"][:].rearrange("(o f) -> o f", o=1))
    beta_row = smp.tile([1, D_OUT], F32, tag=f"bet_{name}")
    nc.sync.dma_start(beta_row[:], ex["beta"][:].rearrange("(o f) -> o f", o=1))

    inv_n = 1.0 / float(c.N_total)
    mean = smp.tile([1, D_OUT], F32, tag=f"mean_{name}")
    nc.vector.tensor_scalar(mean[:], s_row[:], inv_n, None, ALU.mult)
    msq = smp.tile([1, D_OUT], F32, tag=f"msq_{name}")
    nc.vector.tensor_tensor(msq[:], mean[:], mean[:], ALU.mult)
    var = smp.tile([1, D_OUT], F32, tag=f"var_{name}")
    nc.vector.tensor_scalar(var[:], e2_row[:], inv_n, None, ALU.mult)
    nc.vector.tensor_tensor(var[:], var[:], msq[:], ALU.subtract)
    lnv = smp.tile([1, D_OUT], F32, tag=f"lnv_{name}")
    nc.scalar.activation(lnv[:], var[:], AFT.Ln, bias=g.eps_v[:], scale=1.0)
    rstd = smp.tile([1, D_OUT], F32, tag=f"rstd_{name}")
    nc.scalar.activation(rstd[:], lnv[:], AFT.Exp, bias=g.zero_v[:], scale=-0.5)
    a_row = smp.tile([1, D_OUT], F32, tag=f"arow_{name}")
    nc.vector.tensor_tensor(a_row[:], gamma_row[:], rstd[:], ALU.mult)
    ma = smp.tile([1, D_OUT], F32, tag=f"ma_{name}")
    nc.vector.tensor_tensor(ma[:], mean[:], a_row[:], ALU.mult)
    b_row = smp.tile([1, D_OUT], F32, tag=f"brow_{name}")
    nc.vector.tensor_tensor(b_row[:], beta_row[:], ma[:], ALU.subtract)

    # a replicated down d partitions via a K=1 outer product; W' = Wc * a
    arep_ps = psG.tile([d, D_OUT], F32, tag="mps")
    nc.tensor.matmul(arep_ps[:], g.ones_row_f32[0:1, 0:d], a_row[:],
                     start=True, stop=True)
    arep = smp.tile([d, D_OUT], F32, tag=f"arep_{name}")
    nc.vector.tensor_copy(arep[:], arep_ps[:])
    wprime = smp.tile([d, D_OUT], F32, tag=f"wp_{name}")
    nc.vector.tensor_tensor(wprime[:], wc[:], arep[:], ALU.mult)
    wp_stack = g.constp.tile([128, D_OUT], BF16, tag=f"wps_{name}")
    nc.vector.tensor_copy(wp_stack[0:d, :], wprime[:])
    if c.paired:
        nc.sync.dma_start(wp_stack[64:128, :], wp_stack[0:64, :])
    # bias row replicated x4 (bf16) for the K=1 bias-preload matmul
    b_row4 = g.constp.tile([1, 512], BF16, tag=f"br4_{name}")
    for k in range(4):
        nc.vector.tensor_copy(b_row4[:, k * 128:(k + 1) * 128], b_row[:])
    return wp_stack, b_row4


def _p2_item_gen(g, c, ex, xt_store, wp):
    """natural-layout output: psum <- b (K=1 matmul); += x @ W'; Lrelu-evict."""
    nc = g.nc
    wp_stack, b_row4 = wp
    for s0, qc in _chunks(c.Np):
        for z0 in range(0, qc, 8):
            zn = min(8, qc - z0)
            stg = g.p2i.tile([128, 1024], F32, tag="stg_i")
            for t0 in range(0, zn, 4):
                tn = min(4, zn - t0)
                blk0 = s0 + z0 + t0
                nat_ps = g.psI.tile([128, 512], F32, tag="nat_i")
                nc.tensor.matmul(nat_ps[:, 0:tn * 128], g.ones_row_bf[:],
                                 b_row4[:, 0:tn * 128], start=True, stop=False,
                                 skip_group_check=True)
                for i in range(tn):
                    nc.tensor.matmul(
                        nat_ps[:, i * 128:(i + 1) * 128],
                        xt_store[:, (blk0 + i) * 128:(blk0 + i + 1) * 128],
                        wp_stack[:], start=False, stop=True,
                        skip_group_check=True)
                nc.scalar.activation(stg[:, t0 * 128:(t0 + tn) * 128],
                                     nat_ps[:, 0:tn * 128], AFT.Lrelu,
                                     bias=g.zero_col[:], scale=1.0,
                                     alpha=NEG_SLOPE)
                yield True
            dst = ex["out"][s0 * 128:(s0 + qc) * 128, :].rearrange(
                "(p q) e -> p (q e)", p=128)[:, z0 * 128:z0 * 128 + zn * 128]
            nc.sync.dma_start(dst, stg[:, 0:zn * 128])


def _pb_user_gen(g, c, ex, wp):
    """user pass B: re-read x, transpose, natural-layout stage per pair half."""
    nc = g.nc
    wp_stack, b_row4 = wp
    for s0, qc in _chunks(c.Np):
        xch = g.xchp.tile([128, CHUNK_Q * 128], BF16, tag="xch")
        src = ex["x"][s0 * 128:(s0 + qc) * 128, :].rearrange(
            "(p q) d -> p (q d)", p=128)
        nc.gpsimd.dma_start(xch[:, 0:qc * 128], src)
        out_rr = ex["out"][s0 * 256:(s0 + qc) * 256, :].rearrange(
            "(p q) e -> p (q e)", p=128)
        for g0 in range(0, qc, 4):
            ns = min(4, qc - g0)
            xt_ps = g.psUx.tile([128, 512], BF16, tag="xt_u")
            for i in range(ns):
                nc.tensor.transpose(xt_ps[:, i * 128:(i + 1) * 128],
                                    xch[:, (g0 + i) * 128:(g0 + i + 1) * 128],
                                    g.ident_bf[:])
            xt_sb = g.p2u.tile([128, 512], BF16, tag="xtsb_u")
            nc.vector.tensor_copy(xt_sb[:, 0:ns * 128], xt_ps[:, 0:ns * 128])
            stg = g.p2u.tile([128, 1024], F32, tag="stg_u")
            for half in range(2):
                nat_ps = g.psUn.tile([128, 512], F32, tag="nat_u")
                nc.tensor.matmul(nat_ps[:, 0:ns * 128], g.ones_row_bf[:],
                                 b_row4[:, 0:ns * 128], start=True, stop=False,
                                 skip_group_check=True)
                for i in range(ns):
                    nc.tensor.matmul(
                        nat_ps[:, i * 128:(i + 1) * 128],
                        xt_sb[half * 64:(half + 1) * 64, i * 128:(i + 1) * 128],
                        wp_stack[half * 64:(half + 1) * 64, :],
                        start=False, stop=True, skip_group_check=True)
                # this half's real 128-col blocks interleave into staging
                dst = stg[:, 0:2 * ns * 128].rearrange(
                    "p (n two f) -> p n two f", two=2, f=128)[:, :, half, :]
                nc.scalar.activation(
                    dst, nat_ps[:, 0:ns * 128].rearrange("p (n f) -> p n f", f=128),
                    AFT.Lrelu, bias=g.zero_col[:], scale=1.0, alpha=NEG_SLOPE)
            dcols = 2 * ns * 128
            nc.sync.dma_start(out_rr[:, 2 * g0 * 128:2 * g0 * 128 + dcols],
                              stg[:, 0:dcols])
            yield True


# ---------------------------------------------------------------------------
_BUILT = {}


def _get_built():
    if "full" not in _BUILT:
        user = TypeCfg("user", N_USER // NCORES, 64, N_USER)
        item = TypeCfg("item", N_ITEM // NCORES, 128, N_ITEM)
        _BUILT["full"] = (build_kernel(user, item), user, item)
    return _BUILT["full"]


def kernel(x_user, x_item,
           W1_user=None, b1_user=None, W1_item=None, b1_item=None,
           W2_user=None, b2_user=None, W2_item=None, b2_item=None,
           gamma_user=None, beta_user=None, gamma_item=None, beta_item=None,
           _trace=False):
    nc, ucfg, icfg = _get_built()

    def prep(x, cfg):
        x = np.ascontiguousarray(np.asarray(x, np.float32))
        n = x.shape[0] // NCORES
        shards = []
        for i in range(NCORES):
            s = x[i * n:(i + 1) * n].reshape(-1, 128)
            pad = cfg.Np - s.shape[0]
            if pad:
                s = np.concatenate([s, np.zeros((pad, 128), np.float32)], 0)
            shards.append(s)
        return shards

    xu = prep(x_user, ucfg)
    xi = prep(x_item, icfg)
    common = {
        "W1_user": np.asarray(W1_user, np.float32),
        "W2_user": np.asarray(W2_user, np.float32),
        "gamma_user": np.asarray(gamma_user, np.float32),
        "beta_user": np.asarray(beta_user, np.float32),
        "W1_item": np.asarray(W1_item, np.float32),
        "W2_item": np.asarray(W2_item, np.float32),
        "gamma_item": np.asarray(gamma_item, np.float32),
        "beta_item": np.asarray(beta_item, np.float32),
    }
    in_maps = [dict(common, x_user=xu[i], x_item=xi[i]) for i in range(NCORES)]
    res = run_bass_kernel_spmd(nc, in_maps, list(range(NCORES)), trace=_trace)
    nu, ni = N_USER // NCORES, N_ITEM // NCORES
    out_user = np.concatenate(
        [res.results[i]["out_user"][:nu] for i in range(NCORES)], 0)
    out_item = np.concatenate(
        [res.results[i]["out_item"][:ni] for i in range(NCORES)], 0)
    if _trace:
        kernel.last_exec_time_ns = res.exec_time_ns
    return (out_user, out_item)


# revision 32
# speedup vs baseline: 1.8666x; 1.1789x over previous
"""Trainium2 Bass kernel for nn_ActivatedHeteroLinear (moe_routing, 8 cores).

Math: per type t in {user, item}:
    h = (x @ W1 + b1) @ W2 + b2 = x @ Wc + c        (Wc = W1@W2)
    BatchNorm (training mode) is shift-invariant -> the bias c cancels.
    out = LeakyReLU(a * u + b) = LeakyReLU(x @ W' + b),
    W' = Wc * a (columns scaled), a = gamma*rsqrt(var+eps), b = beta - mean*a
    mean = (m @ Wc)/N with m = sum_rows(x); E[u^2] = diag(Wc^T G Wc)/N with
    G = x^T x; var = E[u^2] - mean^2.  Sync-BN = one 66KB AllReduce of
    [G | m] per type.

Schedule:
    setup Wc -> item-p1 (G, m, SBUF-resident xT store bf16)
             -> kick AR_item -> user-pA (G, m only; overlaps AR_item)
             -> kick AR_user -> stats (row layout; fold a into W', b into a
                bias row) -> item-p2 interleaved with user-pB.

p1 per [128 rows, 128 feat] subtile: DMA-cast x f32->bf16 in 1MB chunks
    (packed: partition p holds qc consecutive rows); G += x^T x (PE);
    xT = transpose(x) (PE -> bf16 psum); one ones^T @ slab matmul per
    4-subtile group accumulates m; evict xT -> xT store (DVE).
p2 per 4-subtile group (two-stage chain, natural layout, NO transpose-back):
    psum <- bias row b broadcast (one K=1 matmul, start=True)
    psum += xT_s^T @ W'  per subtile (N=128, accumulate)
    stg  <- LeakyReLU(psum)   (one ScalarE Lrelu = the eviction itself)
    DMA out (contiguous).
user pB re-reads x_user, re-transposes, then the same natural-layout stage;
its even/odd pair halves write interleaved 128-col staging blocks.

x_user [50000,64]/core is viewed host-side as row pairs [25000,128] so both
types share the d=128 path; user W' is stacked [W';W'] and its natural
matmuls use K=64 halves; the pair-G/pair-m fold after the AllReduce.
Rows are zero-padded host-side to a multiple of 128 (zero rows contribute
nothing to G or m).
"""
import sys

for _p in ("/opt/trn_rl_repo",):
    if _p not in sys.path:
        sys.path.insert(0, _p)

import numpy as np

import concourse.mybir as mybir
import concourse.tile as tile
from concourse import bacc
from concourse.masks import make_identity
from concourse.bass_utils import run_bass_kernel_spmd

F32 = mybir.dt.float32
BF16 = mybir.dt.bfloat16
ALU = mybir.AluOpType
AFT = mybir.ActivationFunctionType

NCORES = 8
EPS = 1e-5
NEG_SLOPE = 0.01

N_USER, N_ITEM = 400000, 600000
HID, D_OUT = 256, 128

CHUNK_Q = 16          # packed subtiles per input DMA chunk (1 MB f32)
P2_START_PURE = 6     # item-p2 chunks before user-pB interleave starts


def _ceil_to(x, m):
    return (x + m - 1) // m * m


class TypeCfg:
    def __init__(self, name, n_rows_core, d_in, n_total_rows):
        self.name = name
        self.d_in = d_in                        # true d_in: 64 or 128
        self.paired = d_in == 64
        self.rpp = 2 if self.paired else 1      # real rows per packed row
        assert n_rows_core % self.rpp == 0
        self.Np = _ceil_to(n_rows_core // self.rpp, 128)  # padded packed rows
        self.n_rows_core = n_rows_core
        self.n_out_pad = self.Np * self.rpp
        self.N_total = n_total_rows


def _chunks(Np):
    nsub = Np // 128
    out, s = [], 0
    while s < nsub:
        qc = min(CHUNK_Q, nsub - s)
        out.append((s, qc))
        s += qc
    return out


class _Ctx:
    pass


def build_kernel(user, item, ncores=NCORES):
    nc = bacc.Bacc(None, target_bir_lowering=False, num_devices=ncores)
    cfgs = [user, item]

    ext = {}
    for c in cfgs:
        ext[c.name] = {
            "x": nc.declare_dram_parameter(f"x_{c.name}", [c.Np, 128], F32, isOutput=False),
            "W1": nc.declare_dram_parameter(f"W1_{c.name}", [c.d_in, HID], F32, isOutput=False),
            "W2": nc.declare_dram_parameter(f"W2_{c.name}", [HID, D_OUT], F32, isOutput=False),
            "gamma": nc.declare_dram_parameter(f"gamma_{c.name}", [D_OUT], F32, isOutput=False),
            "beta": nc.declare_dram_parameter(f"beta_{c.name}", [D_OUT], F32, isOutput=False),
            "out": nc.declare_dram_parameter(f"out_{c.name}", [c.n_out_pad, D_OUT], F32, isOutput=True),
        }
    ar_in = {c.name: nc.dram_tensor(f"ar_in_{c.name}", [128, 129], F32) for c in cfgs}
    ar_out = {c.name: nc.dram_tensor(f"ar_out_{c.name}", [128, 129], F32, addr_space="Shared")
              for c in cfgs}
    g = _Ctx()
    g.nc, g.rg = nc, [list(range(ncores))]

    with tile.TileContext(nc) as tc:
        g.tc = tc
        with tc.tile_pool(name="const", bufs=1) as constp:
            g.constp = constp
            ident_bf = constp.tile([128, 128], BF16)
            make_identity(nc, ident_bf[:])
            ident_f32 = constp.tile([128, 128], F32)
            make_identity(nc, ident_f32[:])
            ones_f32 = constp.tile([128, 1], F32)
            nc.gpsimd.memset(ones_f32[:], 1.0)
            ones_bf = constp.tile([128, 1], BF16)
            nc.gpsimd.memset(ones_bf[:], 1.0)
            ones_row_bf = constp.tile([1, 128], BF16)
            nc.gpsimd.memset(ones_row_bf[:], 1.0)
            ones_row_f32 = constp.tile([1, 128], F32)
            nc.gpsimd.memset(ones_row_f32[:], 1.0)
            eps_v = constp.tile([1, 1], F32)
            nc.gpsimd.memset(eps_v[:], EPS)
            zero_v = constp.tile([1, 1], F32)
            nc.gpsimd.memset(zero_v[:], 0.0)
            zero_col = constp.tile([128, 1], F32)
            nc.gpsimd.memset(zero_col[:], 0.0)
            g.ident_bf, g.ident_f32 = ident_bf, ident_f32
            g.ones_f32, g.ones_bf = ones_f32, ones_bf
            g.ones_row_bf, g.ones_row_f32 = ones_row_bf, ones_row_f32
            g.eps_v, g.zero_v, g.zero_col = eps_v, zero_v, zero_col

            # ---- setup: Wc = W1 @ W2 per type ----
            g.wc_f32 = {}
            with (
                tc.tile_pool(name="wsetup", bufs=1) as wsp,
                tc.tile_pool(name="wps", bufs=1, space="PSUM") as wps,
            ):
                for c in cfgs:
                    d = c.d_in
                    w1 = wsp.tile([d, HID], F32, tag=f"w1_{c.name}")
                    nc.sync.dma_start(w1[:], ext[c.name]["W1"][:])
                    w2a = wsp.tile([128, D_OUT], F32, tag=f"w2a_{c.name}")
                    w2b = wsp.tile([128, D_OUT], F32, tag=f"w2b_{c.name}")
                    nc.sync.dma_start(w2a[:], ext[c.name]["W2"][0:128, :])
                    nc.sync.dma_start(w2b[:], ext[c.name]["W2"][128:256, :])
                    wc_ps = wps.tile([d, D_OUT], F32, tag="wc")
                    for h, w2h in enumerate((w2a, w2b)):
                        w1t_ps = wps.tile([128, d], F32, tag="w1t")
                        nc.tensor.transpose(w1t_ps[:], w1[:, h * 128:(h + 1) * 128],
                                            ident_f32[0:d, 0:d])
                        w1t = wsp.tile([128, d], F32, tag=f"w1t_{c.name}_{h}")
                        nc.vector.tensor_copy(w1t[:], w1t_ps[:])
                        nc.tensor.matmul(wc_ps[:], w1t[:], w2h[:],
                                         start=(h == 0), stop=(h == 1))
                    wf = constp.tile([d, D_OUT], F32, tag=f"wcf_{c.name}")
                    nc.vector.tensor_copy(wf[:], wc_ps[:])
                    g.wc_f32[c.name] = wf

            _phases(g, user, item, ext, ar_in, ar_out)

    nc.finalize()
    return nc


def _phases(g, user, item, ext, ar_in, ar_out):
    nc, tc = g.nc, g.tc
    with (
        tc.tile_pool(name="xt", bufs=1) as xtp,
        tc.tile_pool(name="xchp", bufs=3) as xchp,
        tc.tile_pool(name="smp", bufs=1) as smp,
    ):
        g.smp, g.xchp = smp, xchp
        xt_item = xtp.tile([128, item.Np], BF16)

        with (
            tc.tile_pool(name="psA", bufs=4, space="PSUM") as psA,
            tc.tile_pool(name="psG", bufs=1, space="PSUM") as psG,
        ):
            g.psA, g.psG = psA, psG
            _p1(g, item, ext[item.name], xt_item)
            _kick_ar(g, item, ar_in[item.name])
            nc.gpsimd.collective_compute("AllReduce", ALU.add, replica_groups=g.rg,
                                         ins=[ar_in[item.name][:]],
                                         outs=[ar_out[item.name][:]])

            _p1(g, user, ext[user.name], None)
            _kick_ar(g, user, ar_in[user.name])
            nc.gpsimd.collective_compute("AllReduce", ALU.add, replica_groups=g.rg,
                                         ins=[ar_in[user.name][:]],
                                         outs=[ar_out[user.name][:]])

            wp_item = _stats(g, item, ext[item.name], ar_out[item.name])
            wp_user = _stats(g, user, ext[user.name], ar_out[user.name])

        # ---- p2: decoupled pools per stream ----
        with (
            tc.tile_pool(name="p2i", bufs=2) as p2i,
            tc.tile_pool(name="p2u", bufs=2) as p2u,
            tc.tile_pool(name="psI", bufs=3, space="PSUM") as psI,
            tc.tile_pool(name="psUx", bufs=2, space="PSUM") as psUx,
            tc.tile_pool(name="psUn", bufs=3, space="PSUM") as psUn,
        ):
            g.p2i, g.p2u = p2i, p2u
            g.psI, g.psUx, g.psUn = psI, psUx, psUn
            ig = _p2_item_gen(g, item, ext[item.name], xt_item, wp_item)
            ug = _pb_user_gen(g, user, ext[user.name], wp_user)
            for _ in range(4 * P2_START_PURE):
                next(ig, None)
            alive = True
            while alive:
                alive = False
                if next(ig, None) is not None:
                    alive = True
                if next(ug, None) is not None:
                    alive = True


def _p1(g, c, ex, xt_store):
    """G + m accumulation; if xt_store is given, also transpose + store xT.
    Two-group software-pipeline skew keeps the PE stream dense."""
    nc = g.nc
    chunks = _chunks(c.Np)
    nsub = c.Np // 128
    n_groups = sum((qc + 3) // 4 for _, qc in chunks)
    full = xt_store is not None
    gm = g.psG.tile([128, 128], F32, tag="gbank")
    g.g_bank = getattr(g, "g_bank", {})
    g.g_bank[c.name] = gm
    g.m_bank = getattr(g, "m_bank", {})
    m_ps = g.psG.tile([1, 512], F32, tag="mps")
    g.m_bank[c.name] = m_ps

    def flush(pend):
        xt_ps, ns, blk0 = pend
        nc.vector.tensor_copy(xt_store[:, blk0 * 128:(blk0 + ns) * 128],
                              xt_ps[:, 0:ns * 128])

    sub_i = 0
    grp_i = 0
    pend = []
    for s0, qc in chunks:
        xch = g.xchp.tile([128, CHUNK_Q * 128], BF16, tag="xch")
        src = ex["x"][s0 * 128:(s0 + qc) * 128, :].rearrange(
            "(p q) d -> p (q d)", p=128)
        nc.gpsimd.dma_start(xch[:, 0:qc * 128], src)   # f32 -> bf16 cast
        for g0 in range(0, qc, 4):
            ns = min(4, qc - g0)
            slab = xch[:, g0 * 128:(g0 + ns) * 128]
            sls = [xch[:, (g0 + i) * 128:(g0 + i + 1) * 128] for i in range(ns)]
            if full:
                xt_ps = g.psA.tile([128, 512], BF16, tag="xtnat")
            else:
                xt_ps = None
            for i in range(ns):
                nc.tensor.matmul(gm[:], sls[i], sls[i],
                                 start=(sub_i == 0), stop=(sub_i == nsub - 1),
                                 skip_group_check=True)
                if full:
                    nc.tensor.transpose(xt_ps[:, i * 128:(i + 1) * 128],
                                        sls[i], g.ident_bf[:])
                sub_i += 1
            nc.tensor.matmul(m_ps[0:1, 0:ns * 128], g.ones_bf[:], slab,
                             start=(grp_i == 0), stop=(grp_i == n_groups - 1),
                             skip_group_check=True)
            grp_i += 1
            if not full:
                continue
            if len(pend) == 2:
                flush(pend.pop(0))
            pend.append((xt_ps, ns, s0 + g0))
    for p in pend:
        flush(p)
    assert sub_i == nsub and grp_i == n_groups


def _kick_ar(g, c, ar_in):
    nc = g.nc
    gm = g.g_bank[c.name]
    g_sb = g.smp.tile([128, 128], F32, tag=f"gsb_{c.name}")
    nc.vector.tensor_copy(g_sb[:], gm[:])
    nc.sync.dma_start(ar_in[:, 0:128], g_sb[:])
    # fold the [1,512] m blocks, transpose to a column
    m_row4 = g.smp.tile([1, 512], F32, tag=f"mrow4_{c.name}")
    nc.vector.tensor_copy(m_row4[:], g.m_bank[c.name][:])
    m_row = g.smp.tile([1, 128], F32, tag=f"mrow_{c.name}")
    nc.vector.tensor_tensor(m_row[:], m_row4[:, 0:128], m_row4[:, 128:256], ALU.add)
    nc.vector.tensor_tensor(m_row[:], m_row[:], m_row4[:, 256:384], ALU.add)
    nc.vector.tensor_tensor(m_row[:], m_row[:], m_row4[:, 384:512], ALU.add)
    mt_ps = g.psG.tile([128, 4], F32, tag="mps")
    nc.tensor.transpose(mt_ps[:, 0:1], m_row[:], g.ident_f32[0:1, 0:1])
    m_col = g.smp.tile([128, 1], F32, tag=f"mcol_{c.name}")
    nc.vector.tensor_copy(m_col[:], mt_ps[:, 0:1])
    nc.sync.dma_start(ar_in[:, 128:129], m_col[:])


def _stats(g, c, ex, ar_out):
    """AllReduced [G|m] -> (W'_stack bf16 [128,128], b_row4 bf16 [1,512]).
    All per-column stats math in row layout on partition 0."""
    nc, d = g.nc, c.d_in
    smp, psG = g.smp, g.psG
    name = c.name
    ar_sb = smp.tile([128, 129], F32, tag=f"arsb_{name}")
    nc.sync.dma_start(ar_sb[:], ar_out[:])

    if c.paired:
        tmp = smp.tile([64, 65], F32, tag=f"fold_{name}")
        nc.sync.dma_start(tmp[:], ar_sb[64:128, 64:129])   # cross-partition
        g_eff = smp.tile([64, 64], F32, tag=f"geff_{name}")
        nc.vector.tensor_tensor(g_eff[:], ar_sb[0:64, 0:64], tmp[:, 0:64], ALU.add)
        m_eff = smp.tile([64, 1], F32, tag=f"meff_{name}")
        nc.vector.tensor_tensor(m_eff[:], ar_sb[0:64, 128:129], tmp[:, 64:65],
                                ALU.add)
        g_ap, m_ap = g_eff[:], m_eff[:]
    else:
        g_ap, m_ap = ar_sb[:, 0:128], ar_sb[:, 128:129]

    wc = g.wc_f32[name]
    t1_ps = psG.tile([d, D_OUT], F32, tag="mps")
    nc.tensor.matmul(t1_ps[:], g_ap, wc[:], start=True, stop=True)
    t1 = smp.tile([d, D_OUT], F32, tag=f"t1_{name}")
    nc.vector.tensor_copy(t1[:], t1_ps[:])
    t2 = smp.tile([d, D_OUT], F32, tag=f"t2_{name}")
    nc.vector.tensor_tensor(t2[:], t1[:], wc[:], ALU.mult)
    e2_ps = psG.tile([1, D_OUT], F32, tag="mps")
    nc.tensor.matmul(e2_ps[:], g.ones_f32[0:d, :], t2[:], start=True, stop=True)
    e2_row = smp.tile([1, D_OUT], F32, tag=f"e2_{name}")
    nc.vector.tensor_copy(e2_row[:], e2_ps[:])
    s_ps = psG.tile([1, D_OUT], F32, tag="mps")
    nc.tensor.matmul(s_ps[:], m_ap, wc[:], start=True, stop=True)
    s_row = smp.tile([1, D_OUT], F32, tag=f"s_{name}")
    nc.vector.tensor_copy(s_row[:], s_ps[:])
    gamma_row = smp.tile([1, D_OUT], F32, tag=f"gam_{name}")
    nc.sync.dma_start(gamma_row[:], ex["gamma"][:].rearrange("(o f) -> o f", o=1))
    beta_row = smp.tile([1, D_OUT], F32, tag=f"bet_{name}")
    nc.sync.dma_start(beta_row[:], ex["beta"][:].rearrange("(o f) -> o f", o=1))

    inv_n = 1.0 / float(c.N_total)
    mean = smp.tile([1, D_OUT], F32, tag=f"mean_{name}")
    nc.vector.tensor_scalar(mean[:], s_row[:], inv_n, None, ALU.mult)
    msq = smp.tile([1, D_OUT], F32, tag=f"msq_{name}")
    nc.vector.tensor_tensor(msq[:], mean[:], mean[:], ALU.mult)
    var = smp.tile([1, D_OUT], F32, tag=f"var_{name}")
    nc.vector.tensor_scalar(var[:], e2_row[:], inv_n, None, ALU.mult)
    nc.vector.tensor_tensor(var[:], var[:], msq[:], ALU.subtract)
    lnv = smp.tile([1, D_OUT], F32, tag=f"lnv_{name}")
    nc.scalar.activation(lnv[:], var[:], AFT.Ln, bias=g.eps_v[:], scale=1.0)
    rstd = smp.tile([1, D_OUT], F32, tag=f"rstd_{name}")
    nc.scalar.activation(rstd[:], lnv[:], AFT.Exp, bias=g.zero_v[:], scale=-0.5)
    a_row = smp.tile([1, D_OUT], F32, tag=f"arow_{name}")
    nc.vector.tensor_tensor(a_row[:], gamma_row[:], rstd[:], ALU.mult)
    ma = smp.tile([1, D_OUT], F32, tag=f"ma_{name}")
    nc.vector.tensor_tensor(ma[:], mean[:], a_row[:], ALU.mult)
    b_row = smp.tile([1, D_OUT], F32, tag=f"brow_{name}")
    nc.vector.tensor_tensor(b_row[:], beta_row[:], ma[:], ALU.subtract)

    # a replicated down d partitions via a K=1 outer product; W' = Wc * a
    arep_ps = psG.tile([d, D_OUT], F32, tag="mps")
    nc.tensor.matmul(arep_ps[:], g.ones_row_f32[0:1, 0:d], a_row[:],
                     start=True, stop=True)
    arep = smp.tile([d, D_OUT], F32, tag=f"arep_{name}")
    nc.vector.tensor_copy(arep[:], arep_ps[:])
    wprime = smp.tile([d, D_OUT], F32, tag=f"wp_{name}")
    nc.vector.tensor_tensor(wprime[:], wc[:], arep[:], ALU.mult)
    wp_stack = g.constp.tile([128, D_OUT], BF16, tag=f"wps_{name}")
    nc.vector.tensor_copy(wp_stack[0:d, :], wprime[:])
    if c.paired:
        nc.sync.dma_start(wp_stack[64:128, :], wp_stack[0:64, :])
    # bias row replicated x4 (bf16) for the K=1 bias-preload matmul
    b_row4 = g.constp.tile([1, 512], BF16, tag=f"br4_{name}")
    for k in range(4):
        nc.vector.tensor_copy(b_row4[:, k * 128:(k + 1) * 128], b_row[:])
    return wp_stack, b_row4


def _p2_item_gen(g, c, ex, xt_store, wp):
    """natural-layout output: psum <- b (K=1 matmul); += x @ W'; Lrelu-evict."""
    nc = g.nc
    wp_stack, b_row4 = wp
    for s0, qc in _chunks(c.Np):
        for z0 in range(0, qc, 8):
            zn = min(8, qc - z0)
            stg = g.p2i.tile([128, 1024], F32, tag="stg_i")
            for t0 in range(0, zn, 4):
                tn = min(4, zn - t0)
                blk0 = s0 + z0 + t0
                nat_ps = g.psI.tile([128, 512], F32, tag="nat_i")
                nc.tensor.matmul(nat_ps[:, 0:tn * 128], g.ones_row_bf[:],
                                 b_row4[:, 0:tn * 128], start=True, stop=False,
                                 skip_group_check=True)
                for i in range(tn):
                    nc.tensor.matmul(
                        nat_ps[:, i * 128:(i + 1) * 128],
                        xt_store[:, (blk0 + i) * 128:(blk0 + i + 1) * 128],
                        wp_stack[:], start=False, stop=True,
                        skip_group_check=True)
                nc.scalar.activation(stg[:, t0 * 128:(t0 + tn) * 128],
                                     nat_ps[:, 0:tn * 128], AFT.Lrelu,
                                     bias=g.zero_col[:], scale=1.0,
                                     alpha=NEG_SLOPE)
                yield True
            dst = ex["out"][s0 * 128:(s0 + qc) * 128, :].rearrange(
                "(p q) e -> p (q e)", p=128)[:, z0 * 128:z0 * 128 + zn * 128]
            nc.sync.dma_start(dst, stg[:, 0:zn * 128])


def _pb_user_gen(g, c, ex, wp):
    """user pass B: re-read x, transpose, natural-layout stage per pair half."""
    nc = g.nc
    wp_stack, b_row4 = wp
    for s0, qc in _chunks(c.Np):
        xch = g.xchp.tile([128, CHUNK_Q * 128], BF16, tag="xch")
        src = ex["x"][s0 * 128:(s0 + qc) * 128, :].rearrange(
            "(p q) d -> p (q d)", p=128)
        nc.gpsimd.dma_start(xch[:, 0:qc * 128], src)
        out_rr = ex["out"][s0 * 256:(s0 + qc) * 256, :].rearrange(
            "(p q) e -> p (q e)", p=128)
        for g0 in range(0, qc, 4):
            ns = min(4, qc - g0)
            xt_ps = g.psUx.tile([128, 512], BF16, tag="xt_u")
            for i in range(ns):
                nc.tensor.transpose(xt_ps[:, i * 128:(i + 1) * 128],
                                    xch[:, (g0 + i) * 128:(g0 + i + 1) * 128],
                                    g.ident_bf[:])
            xt_sb = g.p2u.tile([128, 512], BF16, tag="xtsb_u")
            nc.vector.tensor_copy(xt_sb[:, 0:ns * 128], xt_ps[:, 0:ns * 128])
            stg = g.p2u.tile([128, 1024], F32, tag="stg_u")
            for half in range(2):
                nat_ps = g.psUn.tile([128, 512], F32, tag="nat_u")
                nc.tensor.matmul(nat_ps[:, 0:ns * 128], g.ones_row_bf[:],
                                 b_row4[:, 0:ns * 128], start=True, stop=False,
                                 skip_group_check=True)
                for i in range(ns):
                    nc.tensor.matmul(
                        nat_ps[:, i * 128:(i + 1) * 128],
                        xt_sb[half * 64:(half + 1) * 64, i * 128:(i + 1) * 128],
                        wp_stack[half * 64:(half + 1) * 64, :],
                        start=False, stop=True, skip_group_check=True)
                # this half's real 128-col blocks interleave into staging
                dst = stg[:, 0:2 * ns * 128].rearrange(
                    "p (n two f) -> p n two f", two=2, f=128)[:, :, half, :]
                nc.scalar.activation(
                    dst, nat_ps[:, 0:ns * 128].rearrange("p (n f) -> p n f", f=128),
                    AFT.Lrelu, bias=g.zero_col[:], scale=1.0, alpha=NEG_SLOPE)
            dcols = 2 * ns * 128
            nc.sync.dma_start(out_rr[:, 2 * g0 * 128:2 * g0 * 128 + dcols],
                              stg[:, 0:dcols])
            yield True


# ---------------------------------------------------------------------------
_BUILT = {}


def _get_built():
    if "full" not in _BUILT:
        user = TypeCfg("user", N_USER // NCORES, 64, N_USER)
        item = TypeCfg("item", N_ITEM // NCORES, 128, N_ITEM)
        _BUILT["full"] = (build_kernel(user, item), user, item)
    return _BUILT["full"]


def kernel(x_user, x_item,
           W1_user=None, b1_user=None, W1_item=None, b1_item=None,
           W2_user=None, b2_user=None, W2_item=None, b2_item=None,
           gamma_user=None, beta_user=None, gamma_item=None, beta_item=None,
           _trace=False):
    nc, ucfg, icfg = _get_built()

    def prep(x, cfg):
        x = np.ascontiguousarray(np.asarray(x, np.float32))
        n = x.shape[0] // NCORES
        shards = []
        for i in range(NCORES):
            s = x[i * n:(i + 1) * n].reshape(-1, 128)
            pad = cfg.Np - s.shape[0]
            if pad:
                s = np.concatenate([s, np.zeros((pad, 128), np.float32)], 0)
            shards.append(s)
        return shards

    xu = prep(x_user, ucfg)
    xi = prep(x_item, icfg)
    common = {
        "W1_user": np.asarray(W1_user, np.float32),
        "W2_user": np.asarray(W2_user, np.float32),
        "gamma_user": np.asarray(gamma_user, np.float32),
        "beta_user": np.asarray(beta_user, np.float32),
        "W1_item": np.asarray(W1_item, np.float32),
        "W2_item": np.asarray(W2_item, np.float32),
        "gamma_item": np.asarray(gamma_item, np.float32),
        "beta_item": np.asarray(beta_item, np.float32),
    }
    in_maps = [dict(common, x_user=xu[i], x_item=xi[i]) for i in range(NCORES)]
    res = run_bass_kernel_spmd(nc, in_maps, list(range(NCORES)), trace=_trace)
    nu, ni = N_USER // NCORES, N_ITEM // NCORES
    out_user = np.concatenate(
        [res.results[i]["out_user"][:nu] for i in range(NCORES)], 0)
    out_item = np.concatenate(
        [res.results[i]["out_item"][:ni] for i in range(NCORES)], 0)
    if _trace:
        kernel.last_exec_time_ns = res.exec_time_ns
    return (out_user, out_item)
